# revision 1
# baseline (speedup 1.0000x reference)
"""ComplexPolarAttention Trainium2 kernel (8-core SPMD, row-sharded).

Math (matching the reference):
  c = mag*cos(phase); s = mag*sin(phase)
  scores = c@c.T + s@s.T + bias     (bias: sparse edge scatter, last-dup-wins)
  attn = softmax(scores, axis=1)
  out = (attn@mag, attn@phase)

Host precomputes everything elementwise-cheap: trig features packed
transposed as xt [128 feat, 8192 nodes] f16, the PV value matrix
mp [128, 64*132] bf16 ([mag|phase|ones] per key chunk), and scalar edge
scores es = edge_attr@W.sum(0)+b.sum().

Device per core (1024 query rows), qb-major: 4 query blocks x 16 groups
(group = 4 key chunks x 256 queries = [128 dst, 1024] score tile):

  PE:     S^T group tile = xt_kc.T @ xtq  (f16, PSUM f32)
  quads q=0,1 (dst chunks 0..31, multiplicative bias
               exp(S+B) = exp(S) * M, M = exp(B) dense from HOST):
     ACT exp straight from PSUM -> p_raw; dense M tiles stream in by
     DMA during the (otherwise idle) steady state; one quad-wide
     bf16 TT mult p = p_raw * M on DVE (2x perf mode, 573ns/group)
  quads q=2,3 (additive bias):
     GpSimd scatters es into dense tiles; DVE tmp = S^T + bias
     (f32 SBUF); ACT exp over the whole quad (FD=4096) -> p
  PE:     pv[128 q, 129] += p_chunk.T @ [mag|phase|ones]  (col 128 =
          softmax denominator); epilogue divides and DMAs out.

QK matmuls for quad q+1 are emitted before PV matmuls of quad q so the
tensor engine never stalls on the exp. The class split balances engines:
ACT ~74 (exp), DVE ~72 (adds + mults), GpSimd ~37, PE ~61.
"""
import os
import sys

sys.path.insert(0, "/opt/trn_rl_repo")

# The NTFF profile hook module is missing from this image's antenv package;
# bass_utils imports it unconditionally when tracing. Create it if absent so
# BASS_TRACE=1 works (degrades silently if dirs are read-only).
_HOOK_SRC = '''_hook = None

def set_axon_ntff_profile_hook(hook):
    global _hook
    _hook = hook

def get_axon_ntff_profile_hook():
    return _hook
'''
for _d in ("/opt/trn_rl_repo/antenv", "/root/.axon_site/_ro/trn_rl_repo/antenv"):
    try:
        _p = os.path.join(_d, "axon_hooks.py")
        if os.path.isdir(_d) and not os.path.exists(_p):
            with open(_p, "w") as _f:
                _f.write(_HOOK_SRC)
    except OSError:
        pass

import numpy as np
import ml_dtypes

import concourse.bass as bass
import concourse.mybir as mybir
import concourse.tile as tile
from concourse import bacc
from concourse.bass_utils import run_bass_kernel_spmd

N, D, E, EDGE_DIM = 8192, 64, 262144, 4
CORES = 8
NQ = N // CORES          # 1024 query rows per core
QB_W = 256               # query block width
N_QB = NQ // QB_W        # 4 query blocks per core
KC = 128                 # key chunk (dst) width
N_KC = N // KC           # 64 key chunks
KCG = 4                  # key chunks per scatter/exp group
N_G = N_KC // KCG        # 16 groups per qb
GW = KCG * QB_W          # 1024 = group tile width
QUAD = 4                 # groups per exp/mult batch
N_Q4 = N_G // QUAD       # 4 quads per qb
N_QM = 2                 # mult-class quads per qb (q = 0, 1)
N_GM = N_QM * QUAD       # mult-class groups per qb (dst chunks 0..31)
N_GA = N_G - N_GM        # add-class groups per qb
MPW = 132                # padded [mag|phase|ones] chunk stride

f32 = mybir.dt.float32
f16 = mybir.dt.float16
bf16 = mybir.dt.bfloat16
i16 = mybir.dt.int16
AF = mybir.ActivationFunctionType
ALU = mybir.AluOpType

_cache = {}
LAST_RESULTS = None


def _build(slots_a):
    tot_a = N_QB * N_GA * slots_a
    MW = N_QB * N_GM * GW  # dense multiplier columns (32 groups x 1024)
    nc = bacc.Bacc("TRN2", target_bir_lowering=False, debug=False,
                   num_devices=CORES)
    xt_d = nc.dram_tensor("xt", (128, N), f16, kind="ExternalInput")
    xtq_d = nc.dram_tensor("xtq", (128, NQ), f16, kind="ExternalInput")
    mp_d = nc.dram_tensor("mp", (128, N_KC * MPW), bf16, kind="ExternalInput")
    eidxa_d = nc.dram_tensor("eidxa", (128, tot_a), i16, kind="ExternalInput")
    esba_d = nc.dram_tensor("esba", (128, tot_a), f16, kind="ExternalInput")
    mm_d = nc.dram_tensor("mmul", (128, MW), bf16, kind="ExternalInput")
    out_d = nc.dram_tensor("out", (NQ, 128), f32, kind="ExternalOutput")

    with tile.TileContext(nc) as tc, \
         tc.tile_pool(name="persist", bufs=1) as pers:
        xt = pers.tile([128, N], f16, tag="xt")
        xtq = pers.tile([128, NQ], f16, tag="xtq")
        mp = pers.tile([128, N_KC * MPW], bf16, tag="mp")
        esba = pers.tile([128, tot_a], f16, tag="esba")
        eidxa = pers.tile([128, tot_a], i16, tag="eidxa")

        # Ramp-critical inputs first, spread across the engine DGE queues.
        tqa = tot_a // N_QB
        for qb in range(N_QB):
            nc.gpsimd.dma_start(out=esba[:, qb * tqa:(qb + 1) * tqa],
                                in_=esba_d[:, qb * tqa:(qb + 1) * tqa])
            nc.gpsimd.dma_start(out=eidxa[:, qb * tqa:(qb + 1) * tqa],
                                in_=eidxa_d[:, qb * tqa:(qb + 1) * tqa])
        nc.sync.dma_start(out=xtq[:], in_=xtq_d[:])
        NCH = 16
        for h in range(NCH):
            a, b = h * (N // NCH), (h + 1) * (N // NCH)
            nc.sync.dma_start(out=xt[:, a:b], in_=xt_d[:, a:b])

        with tc.tile_pool(name="qk_ps", bufs=2, space="PSUM") as qkp, \
             tc.tile_pool(name="pv_ps", bufs=2, space="PSUM") as pvp, \
             tc.tile_pool(name="psb", bufs=2) as psbp, \
             tc.tile_pool(name="tmp", bufs=2) as tmpp, \
             tc.tile_pool(name="praw", bufs=2) as prawp, \
             tc.tile_pool(name="mmul", bufs=5) as mmp, \
             tc.tile_pool(name="bias", bufs=6) as biasp, \
             tc.tile_pool(name="epi", bufs=2) as epip:

            # mp chunks + dense multiplier chunks interleaved on the scalar
            # DGE queue in consumption order: PV of group g needs mp chunk
            # g; the mult quads (q=0,1) of qb need their M chunk early.
            m_tiles = {}

            def dma_mchunk(qb, q):
                m = mmp.tile([128, QUAD * GW], bf16, tag="m")
                c0 = (qb * N_QM + q) * QUAD * GW
                nc.scalar.dma_start(out=m[:], in_=mm_d[:, c0:c0 + QUAD * GW])
                m_tiles[(qb, q)] = m

            NMC = 16  # mp chunk h covers groups 4h..4h+3 (kc 16h..16h+15)
            dma_mchunk(0, 0)
            for h in range(4):
                am, bm = h * (N_KC * MPW // NMC), (h + 1) * (N_KC * MPW // NMC)
                nc.scalar.dma_start(out=mp[:, am:bm], in_=mp_d[:, am:bm])
            dma_mchunk(0, 1)
            for h in range(4, NMC):
                am, bm = h * (N_KC * MPW // NMC), (h + 1) * (N_KC * MPW // NMC)
                nc.scalar.dma_start(out=mp[:, am:bm], in_=mp_d[:, am:bm])
            for qb in range(1, N_QB):
                for q in range(N_QM):
                    dma_mchunk(qb, q)

            def emit_qk(qb, q):
                """QK matmuls for one quad; returns the 4 psum tiles."""
                tiles = []
                for gl in range(QUAD):
                    g = q * QUAD + gl
                    qk = qkp.tile([128, GW], f32, tag="qk")
                    for j in range(KCG):
                        kc = g * KCG + j
                        nc.tensor.matmul(
                            out=qk[:, j * QB_W:(j + 1) * QB_W],
                            lhsT=xt[:, kc * 128:(kc + 1) * 128],
                            rhs=xtq[:, qb * QB_W:(qb + 1) * QB_W],
                            start=True, stop=True)
                    tiles.append(qk)
                return tiles

            pend = None
            for qb in range(N_QB):
                p_sb = psbp.tile([128, N_G * GW], bf16, tag="p_sb")
                pv0 = pvp.tile([128, 129], f32, tag="pv0")
                pv1 = pvp.tile([128, 129], f32, tag="pv1")
                for q in range(N_Q4):
                    qk_tiles = pend if pend is not None else emit_qk(qb, q)
                    pend = None
                    if q < N_QM:
                        # multiplicative: exp from PSUM, then one quad-wide
                        # bf16 mult with the host-provided dense M (2x DVE)
                        p_raw = prawp.tile([128, QUAD * GW], bf16,
                                           tag="p_raw")
                        for gl in range(QUAD):
                            nc.scalar.activation(
                                out=p_raw[:, gl * GW:(gl + 1) * GW],
                                in_=qk_tiles[gl][:], func=AF.Exp)
                        nc.vector.tensor_tensor(
                            out=p_sb[:, q * QUAD * GW:(q + 1) * QUAD * GW],
                            in0=p_raw[:], in1=m_tiles[(qb, q)][:],
                            op=ALU.mult)
                    else:
                        tmp = tmpp.tile([128, QUAD * GW], f32, tag="tmp")
                        for gl in range(QUAD):
                            ga = (q - N_QM) * QUAD + gl
                            bias_t = biasp.tile([128, GW], f16, tag="bias_t")
                            off = (qb * N_GA + ga) * slots_a
                            nc.gpsimd.local_scatter(
                                bias_t[:], esba[:, off:off + slots_a],
                                eidxa[:, off:off + slots_a],
                                channels=128, num_elems=GW,
                                num_idxs=slots_a)
                            nc.vector.tensor_tensor(
                                out=tmp[:, gl * GW:(gl + 1) * GW],
                                in0=qk_tiles[gl][:], in1=bias_t[:],
                                op=ALU.add)
                        nc.scalar.activation(
                            out=p_sb[:, q * QUAD * GW:(q + 1) * QUAD * GW],
                            in_=tmp[:], func=AF.Exp)
                    # queue next quad's QK ahead of this quad's PV so the
                    # tensor engine never waits on the exp
                    if q + 1 < N_Q4:
                        pend = emit_qk(qb, q + 1)
                    elif qb + 1 < N_QB:
                        pend = emit_qk(qb + 1, 0)
                    for gl in range(QUAD):
                        g = q * QUAD + gl
                        for j in range(KCG):
                            kc = g * KCG + j
                            col = g * GW + j * QB_W
                            for qs, pv in ((0, pv0), (1, pv1)):
                                nc.tensor.matmul(
                                    out=pv[:],
                                    lhsT=p_sb[:, col + qs * 128:
                                              col + (qs + 1) * 128],
                                    rhs=mp[:, kc * MPW:kc * MPW + 2 * D + 1],
                                    start=(kc == 0), stop=(kc == N_KC - 1))
                for qs, pv in ((0, pv0), (1, pv1)):
                    rec = epip.tile([128, 1], f32, tag=f"rec{qs}")
                    nc.vector.reciprocal(out=rec[:], in_=pv[:, 128:129])
                    o_t = epip.tile([128, 128], f32, tag=f"o_t{qs}")
                    nc.vector.tensor_scalar(o_t[:], pv[:, 0:128], rec[:], None,
                                            ALU.mult)
                    r0 = qb * QB_W + qs * 128
                    nc.sync.dma_start(out=out_d[r0:r0 + 128, :], in_=o_t[:])

    nc.compile()
    return nc


def _prep_edges(src, dst, vals, n_groups, g_local):
    """Bucket pre-deduped additive-class edges into scatter layout.

    cell = (core, qb, g_local, p): qb = src query block, p = dst % 128;
    scattered column inside the [128, 1024] group tile is
    ((dst % 512) // 128) * 256 + src % 256."""
    core = src // NQ
    qb = (src % NQ) // QB_W
    p = dst % 128
    col = ((dst % (KCG * KC)) // KC) * QB_W + (src % QB_W)

    cell = ((core * N_QB + qb) * n_groups + g_local) * 128 + p
    o2 = np.argsort(cell, kind="stable")
    cell_s = cell[o2]
    first = np.r_[True, cell_s[1:] != cell_s[:-1]]
    run_id = np.cumsum(first) - 1
    run_start = np.flatnonzero(first)
    slot = np.arange(len(cell_s)) - run_start[run_id]
    slots = int(max(int(slot.max()) + 1 if len(slot) else 1, 4))
    slots = (slots + 1) // 2 * 2  # even

    tot = N_QB * n_groups * slots
    eidx = np.full((CORES, 128, tot), -1, dtype=np.int16)
    esb = np.zeros((CORES, 128, tot), dtype=np.float16)
    cs, qbs, gs, ps = core[o2], qb[o2], g_local[o2], p[o2]
    off = (qbs * n_groups + gs) * slots + slot
    eidx[cs, ps, off] = col[o2].astype(np.int16)
    esb[cs, ps, off] = vals[o2].astype(np.float16)
    return eidx, esb, slots


def kernel(mag, phase, edge_index, edge_attr, W, b):
    global LAST_RESULTS
    mag = np.asarray(mag, dtype=np.float32)
    phase = np.asarray(phase, dtype=np.float32)
    W = np.asarray(W, dtype=np.float32)
    bv = np.asarray(b, dtype=np.float32)

    # trig features, packed transposed: xt[[cos|sin] x d, node]
    c = (mag * np.cos(phase)).astype(np.float16)
    s = (mag * np.sin(phase)).astype(np.float16)
    xt = np.ascontiguousarray(np.concatenate([c.T, s.T], axis=0))  # [128, N]

    # PV value matrix per key chunk: [mag | phase | 1 | pad] stride 132
    mp = np.zeros((128, N_KC, MPW), dtype=np.float32)
    mp[:, :, 0:D] = mag.reshape(N_KC, 128, D).transpose(1, 0, 2)
    mp[:, :, D:2 * D] = phase.reshape(N_KC, 128, D).transpose(1, 0, 2)
    mp[:, :, 2 * D] = 1.0
    mp = mp.reshape(128, N_KC * MPW).astype(ml_dtypes.bfloat16)

    # scalar edge scores: sum_h (edge_attr @ W.T + b)[:, h]; dedup last-wins
    es_all = (np.asarray(edge_attr, dtype=np.float64) @
              W.astype(np.float64).sum(axis=0) + bv.astype(np.float64).sum())
    src = np.asarray(edge_index[0], dtype=np.int64)
    dst = np.asarray(edge_index[1], dtype=np.int64)
    keys = src * N + dst
    order = np.argsort(keys, kind="stable")
    ks = keys[order]
    run_last = np.flatnonzero(np.r_[ks[1:] != ks[:-1], True])
    kept = order[run_last]  # stable sort => last occurrence per duplicate key
    src, dst, es = src[kept], dst[kept], es_all[kept]

    # class split by dst chunk: groups 0..N_GM-1 multiplicative (dense M
    # from host), rest additive (device scatter + add)
    g = dst // (KCG * KC)
    is_m = g < N_GM

    # dense multiplier M: [core][128 p, (qb, q, gl, j, srccol) cols]
    sm, dm, em = src[is_m], dst[is_m], np.exp(es[is_m])
    mmul = np.ones((CORES, 128, N_QB * N_GM * GW), dtype=np.float32)
    colm = ((dm // (KCG * KC)) * GW + ((dm % (KCG * KC)) // KC) * QB_W +
            (sm % QB_W))
    qbm = (sm % NQ) // QB_W
    mmul[sm // NQ, dm % 128, qbm * (N_GM * GW) + colm] = em
    mmul = mmul.astype(ml_dtypes.bfloat16)

    eidxa, esba, slots_a = _prep_edges(
        src[~is_m], dst[~is_m], es[~is_m], N_GA, g[~is_m] - N_GM)

    if slots_a not in _cache:
        _cache[slots_a] = _build(slots_a)
    nc = _cache[slots_a]

    in_maps = []
    for cid in range(CORES):
        in_maps.append({
            "xt": xt,
            "xtq": np.ascontiguousarray(xt[:, cid * NQ:(cid + 1) * NQ]),
            "mp": mp,
            "eidxa": np.ascontiguousarray(eidxa[cid]),
            "esba": np.ascontiguousarray(esba[cid]),
            "mmul": np.ascontiguousarray(mmul[cid]),
        })
    res = run_bass_kernel_spmd(nc, in_maps, core_ids=list(range(CORES)))
    LAST_RESULTS = res

    new_mag = np.empty((N, D), dtype=np.float32)
    new_phase = np.empty((N, D), dtype=np.float32)
    for cid in range(CORES):
        o = res.results[cid]["out"]
        new_mag[cid * NQ:(cid + 1) * NQ] = o[:, 0:D]
        new_phase[cid * NQ:(cid + 1) * NQ] = o[:, D:2 * D]
    return new_mag, new_phase



# revision 4
# speedup vs baseline: 1.4333x; 1.4333x over previous
"""ComplexPolarAttention Trainium2 kernel (8-core SPMD, row-sharded).

Math (matching the reference):
  c = mag*cos(phase); s = mag*sin(phase)
  scores = c@c.T + s@s.T + bias     (bias: sparse edge scatter, last-dup-wins)
  attn = softmax(scores, axis=1)
  out = (attn@mag, attn@phase)

Design (v2, all-multiplicative bias):
  exp(S+B) = exp(S) * M with M = exp(B) provided DENSE from the host in
  bf16 (1.0 everywhere except the ~0.4% edge cells). Host precomputes
  trig features packed transposed as xt [128 feat, 8192 nodes] f16, the
  PV value matrix mp [128, 64*132] bf16 ([mag|phase|ones] per key
  chunk), and M per core [128, 65536] bf16.

Device per core (1024 query rows), per group g (= 4 key chunks x 256
queries = [128 dst, 1024] score tile):
  PE:   S^T group tile = xt_kc.T @ xtq   (f16, PSUM f32)
  ACT:  p_raw = exp(S^T) straight from PSUM -> bf16 SBUF (the pacing
        engine: 64 x ~1.05us back-to-back)
  DVE:  p = p_raw * M_g   (bf16 2x mode)
  PE:   pv[128 q, 129] += p_chunk.T @ [mag|phase|ones]  (col 128 =
        softmax denominator); epilogue divides and DMAs out.

QK matmuls for quad q+1 are emitted before PV matmuls of quad q so the
tensor engine never stalls on the exp. The scalar (ACT) queue carries
NO DMA configs (they serialize the sequencer ahead of the first exp);
M streams on the gpsimd DGE queue (25ns/config), xt/xtq/out on sync,
mp + late xt chunks on the vector queue.
"""
import os
import sys

sys.path.insert(0, "/opt/trn_rl_repo")

# The NTFF profile hook module is missing from this image's antenv package;
# bass_utils imports it unconditionally when tracing. Create it if absent so
# BASS_TRACE=1 works (degrades silently if dirs are read-only).
_HOOK_SRC = '''_hook = None

def set_axon_ntff_profile_hook(hook):
    global _hook
    _hook = hook

def get_axon_ntff_profile_hook():
    return _hook
'''
for _d in ("/opt/trn_rl_repo/antenv", "/root/.axon_site/_ro/trn_rl_repo/antenv"):
    try:
        _p = os.path.join(_d, "axon_hooks.py")
        if os.path.isdir(_d) and not os.path.exists(_p):
            with open(_p, "w") as _f:
                _f.write(_HOOK_SRC)
    except OSError:
        pass

import numpy as np
import ml_dtypes

import concourse.bass as bass
import concourse.mybir as mybir
import concourse.tile as tile
from concourse import bacc
from concourse.bass_utils import run_bass_kernel_spmd

N, D, E, EDGE_DIM = 8192, 64, 262144, 4
CORES = 8
NQ = N // CORES          # 1024 query rows per core
QB_W = 256               # query block width
N_QB = NQ // QB_W        # 4 query blocks per core
KC = 128                 # key chunk (dst) width
N_KC = N // KC           # 64 key chunks
KCG = 4                  # key chunks per group
N_G = N_KC // KCG        # 16 groups per qb
GW = KCG * QB_W          # 1024 = group tile width
QUAD = 4                 # groups per QK-emission batch
N_Q4 = N_G // QUAD       # 4 quads per qb
MPW = 132                # padded [mag|phase|ones] chunk stride
MW = N_QB * N_G * GW     # dense multiplier columns per core (65536)

f32 = mybir.dt.float32
f16 = mybir.dt.float16
bf16 = mybir.dt.bfloat16
AF = mybir.ActivationFunctionType
ALU = mybir.AluOpType

_cache = {}
LAST_RESULTS = None


def _build():
    nc = bacc.Bacc("TRN2", target_bir_lowering=False, debug=False,
                   num_devices=CORES)
    xt_d = nc.dram_tensor("xt", (128, N), f16, kind="ExternalInput")
    xtq_d = nc.dram_tensor("xtq", (128, NQ), f16, kind="ExternalInput")
    mp_d = nc.dram_tensor("mp", (128, N_KC * MPW), bf16, kind="ExternalInput")
    mm_d = nc.dram_tensor("mmul", (128, MW), bf16, kind="ExternalInput")
    out_d = nc.dram_tensor("out", (NQ, 128), f32, kind="ExternalOutput")

    with tile.TileContext(nc) as tc, \
         tc.tile_pool(name="persist", bufs=1) as pers:
        xt = pers.tile([128, N], f16, tag="xt")
        xtq = pers.tile([128, NQ], f16, tag="xtq")
        mp = pers.tile([128, N_KC * MPW], bf16, tag="mp")

        # Ramp-critical inputs first; NOTHING on the scalar queue (DGE
        # configs there would serialize the ACT sequencer ahead of the
        # first exp). sync queue: xtq + xt + half of M + out; gpsimd
        # queue: mp + other half of M. 2KB+ per-partition descriptors.
        nc.sync.dma_start(out=xtq[:], in_=xtq_d[:])
        NCH = 4
        CW = N // NCH
        for h in range(NCH):
            a, b = h * CW, (h + 1) * CW
            nc.sync.dma_start(out=xt[:, a:b], in_=xt_d[:, a:b])
        NMC = 2
        MCW = N_KC * MPW // NMC
        for h in range(NMC):
            a, b = h * MCW, (h + 1) * MCW
            nc.gpsimd.dma_start(out=mp[:, a:b], in_=mp_d[:, a:b])

        with tc.tile_pool(name="qk_ps", bufs=3, space="PSUM") as qkp, \
             tc.tile_pool(name="pv_ps", bufs=1, space="PSUM") as pvp, \
             tc.tile_pool(name="mmul", bufs=5) as mmp, \
             tc.tile_pool(name="praw", bufs=4) as prawp, \
             tc.tile_pool(name="psb", bufs=4) as psbp, \
             tc.tile_pool(name="epi", bufs=2) as epip:

            # M quad tiles [128, 4096] stream on the sync/gpsimd DGE
            # queues (alternating) in consumption order; the 5-deep pool
            # self-paces prefetch (~5MB / ~21us of runway).
            m_tiles = {}

            def dma_m(qb, q):
                m = mmp.tile([128, QUAD * GW], bf16, tag="m")
                c0 = (qb * N_Q4 + q) * QUAD * GW
                eng = nc.sync if (qb * N_Q4 + q) % 2 == 0 else nc.gpsimd
                eng.dma_start(out=m[:], in_=mm_d[:, c0:c0 + QUAD * GW])
                m_tiles[(qb, q)] = m

            for qb in range(N_QB):
                for q in range(N_Q4):
                    dma_m(qb, q)

            def emit_qk(qb, q):
                """QK matmuls for one quad; returns the 4 psum tiles."""
                tiles = []
                for gl in range(QUAD):
                    g = q * QUAD + gl
                    qk = qkp.tile([128, GW], f32, tag="qk")
                    for j in range(KCG):
                        kc = g * KCG + j
                        nc.tensor.matmul(
                            out=qk[:, j * QB_W:(j + 1) * QB_W],
                            lhsT=xt[:, kc * 128:(kc + 1) * 128],
                            rhs=xtq[:, qb * QB_W:(qb + 1) * QB_W],
                            start=True, stop=True)
                    tiles.append(qk)
                return tiles

            pend = None
            for qb in range(N_QB):
                pv0 = pvp.tile([128, 129], f32, tag="pv0")
                pv1 = pvp.tile([128, 129], f32, tag="pv1")
                for q in range(N_Q4):
                    qk_tiles = pend if pend is not None else emit_qk(qb, q)
                    pend = None
                    m_q = m_tiles[(qb, q)]
                    psb_tiles = []
                    for gl in range(QUAD):
                        p_raw = prawp.tile([128, GW], bf16, tag="p_raw")
                        nc.scalar.activation(out=p_raw[:], in_=qk_tiles[gl][:],
                                             func=AF.Exp)
                        p_sb = psbp.tile([128, GW], bf16, tag="p_sb")
                        nc.vector.tensor_tensor(
                            out=p_sb[:], in0=p_raw[:],
                            in1=m_q[:, gl * GW:(gl + 1) * GW],
                            op=ALU.mult)
                        psb_tiles.append(p_sb)
                    # queue next quad's QK ahead of this quad's PV so the
                    # tensor engine never waits on the exp
                    if q + 1 < N_Q4:
                        pend = emit_qk(qb, q + 1)
                    elif qb + 1 < N_QB:
                        pend = emit_qk(qb + 1, 0)
                    for gl in range(QUAD):
                        g = q * QUAD + gl
                        for j in range(KCG):
                            kc = g * KCG + j
                            col = j * QB_W
                            for qs, pv in ((0, pv0), (1, pv1)):
                                nc.tensor.matmul(
                                    out=pv[:],
                                    lhsT=psb_tiles[gl][:, col + qs * 128:
                                                       col + (qs + 1) * 128],
                                    rhs=mp[:, kc * MPW:kc * MPW + 2 * D + 1],
                                    start=(kc == 0), stop=(kc == N_KC - 1))
                for qs, pv in ((0, pv0), (1, pv1)):
                    rec = epip.tile([128, 1], f32, tag=f"rec{qs}")
                    nc.vector.reciprocal(out=rec[:], in_=pv[:, 128:129])
                    o_t = epip.tile([128, 128], f32, tag=f"o_t{qs}")
                    nc.vector.tensor_scalar(o_t[:], pv[:, 0:128], rec[:], None,
                                            ALU.mult)
                    r0 = qb * QB_W + qs * 128
                    nc.sync.dma_start(out=out_d[r0:r0 + 128, :], in_=o_t[:])

    nc.compile()
    return nc


def kernel(mag, phase, edge_index, edge_attr, W, b):
    global LAST_RESULTS
    mag = np.asarray(mag, dtype=np.float32)
    phase = np.asarray(phase, dtype=np.float32)
    W = np.asarray(W, dtype=np.float32)
    bv = np.asarray(b, dtype=np.float32)

    # trig features, packed transposed: xt[[cos|sin] x d, node]
    c = (mag * np.cos(phase)).astype(np.float16)
    s = (mag * np.sin(phase)).astype(np.float16)
    xt = np.ascontiguousarray(np.concatenate([c.T, s.T], axis=0))  # [128, N]

    # PV value matrix per key chunk: [mag | phase | 1 | pad] stride 132
    mp = np.zeros((128, N_KC, MPW), dtype=np.float32)
    mp[:, :, 0:D] = mag.reshape(N_KC, 128, D).transpose(1, 0, 2)
    mp[:, :, D:2 * D] = phase.reshape(N_KC, 128, D).transpose(1, 0, 2)
    mp[:, :, 2 * D] = 1.0
    mp = mp.reshape(128, N_KC * MPW).astype(ml_dtypes.bfloat16)

    # scalar edge scores: sum_h (edge_attr @ W.T + b)[:, h]; dedup last-wins
    es_all = (np.asarray(edge_attr, dtype=np.float64) @
              W.astype(np.float64).sum(axis=0) + bv.astype(np.float64).sum())
    src = np.asarray(edge_index[0], dtype=np.int64)
    dst = np.asarray(edge_index[1], dtype=np.int64)
    keys = src * N + dst
    order = np.argsort(keys, kind="stable")
    ks = keys[order]
    run_last = np.flatnonzero(np.r_[ks[1:] != ks[:-1], True])
    kept = order[run_last]  # stable sort => last occurrence per duplicate key
    src, dst, es = src[kept], dst[kept], es_all[kept]

    # dense multiplier M = exp(bias): [core][128 p, (qb, g, j, srccol) cols]
    em = np.exp(es)
    mmul = np.full((CORES, 128, MW), 0x3F80, dtype=np.uint16)  # bf16 1.0
    mmul = mmul.view(ml_dtypes.bfloat16)
    col = ((dst // (KCG * KC)) * GW + ((dst % (KCG * KC)) // KC) * QB_W +
           (src % QB_W))
    qbi = (src % NQ) // QB_W
    mmul[src // NQ, dst % 128, qbi * (N_G * GW) + col] = \
        em.astype(ml_dtypes.bfloat16)

    if "nc" not in _cache:
        _cache["nc"] = _build()
    nc = _cache["nc"]

    in_maps = []
    for cid in range(CORES):
        in_maps.append({
            "xt": xt,
            "xtq": np.ascontiguousarray(xt[:, cid * NQ:(cid + 1) * NQ]),
            "mp": mp,
            "mmul": mmul[cid],
        })
    res = run_bass_kernel_spmd(nc, in_maps, core_ids=list(range(CORES)))
    LAST_RESULTS = res

    new_mag = np.empty((N, D), dtype=np.float32)
    new_phase = np.empty((N, D), dtype=np.float32)
    for cid in range(CORES):
        o = res.results[cid]["out"]
        new_mag[cid * NQ:(cid + 1) * NQ] = o[:, 0:D]
        new_phase[cid * NQ:(cid + 1) * NQ] = o[:, D:2 * D]
    return new_mag, new_phase
